# revision 9
# baseline (speedup 1.0000x reference)
"""Depthwise 3x3 conv (SAME, channel multiplier 2) on [16,224,224,96] f32,
data-parallel over batch across 8 TRN2 NeuronCores.

Per-core mapping (2 images/core): the conv along H is expressed as a banded
matmul on TensorE — stationary [116,112] band matrix whose 3 diagonals hold
the kernel column k[:, dw, m]; contract dim = 114 input rows (112 out rows +
halo) + 2 bias rows (all-ones coefficients fold the bias add into the PE).
The 3 W-shifts accumulate into PSUM via the moving operand's column offset
(dw*96 in the flattened (w,c) free dim).  f16 operands stream at 1 col/cycle
(2.4 GHz) — 1008 matmuls x 512 cols ~= 218 us tensor floor.

Output is written to HBM as f16 (halves the dominant DMA stream vs f32;
quantization adds ~3e-4 rel err) and upcast to f32 on the host.  PSUM
[112, 512] per (chunk, m) pairs into [112,1024] tiles; DVE/ACT
interleave-copy+cast them (out ch = 2c+m -> stride-2 write) into SBUF groups
of 7 chunks, DMA'd out as contiguous 14KB/partition runs.
"""

import sys

sys.path.insert(0, "/opt/trn_rl_repo")

import numpy as np

B, H, W, C = 16, 224, 224, 96
MULT = 2
NCORES = 8
BPC = B // NCORES  # images per core
M = 112            # output rows per h-tile
KP = 116           # contract partitions: 114 x rows + 2 bias rows
WH = 112           # w-half width
COLS = (WH + 2) * C         # 10944 x-tile cols (1-w halo each side)
CHUNK = 512
NCHUNK = WH * C // CHUNK    # 21
GRP = 7                     # chunks per output DMA group
NGRP = NCHUNK // GRP        # 3

_cache = {}
XDT = "f16"  # input/matmul operand dtype: "f32r" or "f16"


def _build():
    import concourse.bacc as bacc
    import concourse.tile as tile
    from concourse import mybir

    f32 = mybir.dt.float32
    f16 = mybir.dt.float16
    f32r = mybir.dt.float32r if XDT == "f32r" else f16

    nc = bacc.Bacc("TRN2", target_bir_lowering=False, debug=False)
    x_d = nc.dram_tensor("x", [BPC, H, W, C], f32r, kind="ExternalInput")
    bands_d = nc.dram_tensor("bands", [KP, 12 * M], f32r, kind="ExternalInput")
    brows_d = nc.dram_tensor("brows", [MULT, COLS], f32r, kind="ExternalInput")
    out_d = nc.dram_tensor("out", [BPC, H, W, C * MULT], f16, kind="ExternalOutput")

    with tile.TileContext(nc) as tc:
        with (
            tc.tile_pool(name="const", bufs=1) as const,
            tc.tile_pool(name="xp", bufs=3) as xp,
            tc.tile_pool(name="op", bufs=3) as op,
            tc.tile_pool(name="ps", bufs=4, space="PSUM") as ps,
        ):
            band_t = const.tile([KP, 12 * M], f32r)
            nc.sync.dma_start(band_t[0:112, :], bands_d[0:112, :])
            nc.sync.dma_start(band_t[112:KP, :], bands_d[112:KP, :])

            ev = 0  # eviction round-robin DVE/ACT
            ntile = 2 * 2 * BPC
            ti = 0
            for b in range(BPC):
                for ht in range(2):
                    h0 = ht * M
                    hs = 0 if ht == 0 else 110
                    for wh in range(2):
                        w0 = wh * WH
                        ws = 0 if wh == 0 else 110
                        # jk tap offset in tile cols: col = flat + 96*(jk-1) for
                        # wh=0 (tile holds w 0..113), col = flat + 96*(jk+1) for
                        # wh=1 (tile holds w 110..223).  The single out-of-range
                        # (chunk, jk) at each image w-edge is clipped to N=416 —
                        # the dropped 96 columns are exactly the SAME-pad taps.
                        joff = -1 if wh == 0 else 1
                        jorder = (1, 2, 0) if wh == 0 else (1, 0, 2)
                        xt = xp.tile([KP, COLS], f32r)
                        # bias rows first: every chunk's matmul group depends on
                        # them, and they are tiny
                        nc.sync.dma_start(xt[114:KP, :], brows_d[:, :])
                        # x pieces include the 2 halo rows (114 partitions) so a
                        # piece landing unblocks its chunks; the first tile
                        # ladders finely to launch the output stream as early
                        # as possible
                        first = ti == 0
                        last = ti == ntile - 1
                        wsplit = (0, 8, 13, 41, 80, 114) if first else (0, 114)
                        for wa, wb_ in zip(wsplit, wsplit[1:]):
                            nc.sync.dma_start(
                                xt[0:114, wa * C : wb_ * C],
                                x_d[b, hs : hs + 114, ws + wa : ws + wb_, :],
                            )

                        od = out_d[b].rearrange("h w c -> h (w c)")
                        if first:
                            groups = (1, 2, 4, 7, 7)
                        elif last:
                            groups = (7, 7, 4, 2, 1)
                        else:
                            groups = (11, 10)
                        # one og buffer per output DMA group; fine groups at the
                        # head launch the store stream early, fine at the tail
                        # drain early, big in the middle amortize DMA overhead
                        ch = 0
                        for gsz in groups:
                            og = op.tile([M, 11 * CHUNK * MULT], f16, tag="og")
                            gbase = ch
                            for q in range(gsz):
                                n0 = ch * CHUNK
                                pt = ps.tile([M, 2 * CHUNK], f32)
                                for m in range(MULT):
                                    for idx, jk in enumerate(jorder):
                                        bi = ht * 6 + m * 3 + jk
                                        c0 = n0 + 96 * (jk + joff)
                                        p0, p1 = 0, CHUNK
                                        if c0 < 0:
                                            p0, c0 = -c0, 0
                                        elif c0 + CHUNK > COLS:
                                            p1 = COLS - c0
                                        nc.tensor.matmul(
                                            pt[:, m * CHUNK + p0 : m * CHUNK + p1],
                                            band_t[:, bi * M : (bi + 1) * M],
                                            xt[0:KP, c0 : c0 + (p1 - p0)],
                                            start=(idx == 0),
                                            stop=(idx == 2),
                                        )
                                src = pt[:, :].rearrange("p (m n) -> p n m", m=2)
                                dst = og[
                                    :, q * 1024 : (q + 1) * 1024
                                ].rearrange("p (n m) -> p n m", m=2)
                                if ev % 2 == 0:
                                    nc.vector.tensor_copy(dst, src)
                                else:
                                    nc.scalar.copy(dst, src)
                                ev += 1
                                ch += 1
                            cb = w0 * C * MULT + gbase * CHUNK * MULT
                            glen = gsz * CHUNK * MULT
                            nc.scalar.dma_start(
                                od[h0 : h0 + M, cb : cb + glen], og[:, 0:glen]
                            )
                        ti += 1
    nc.compile()
    return nc


def _host_consts(kern, bias):
    kk = np.asarray(kern, np.float32).reshape(3, 3, MULT)  # [dh, dw, m]
    bands = np.zeros((12, KP, M), np.float32)
    for ht in range(2):
        for m in range(MULT):
            for jk in range(3):
                band = bands[ht * 6 + m * 3 + jk]
                for i in range(3):
                    if ht == 0:
                        # tile row k holds x row h=k; out j needs rows j+i-1
                        ks = np.arange(M) + i - 1
                    else:
                        # tile row k holds x row h=110+k; out h=112+j reads
                        # h_in=111+j+i -> k=1+j+i (h_in=224 dropped: SAME pad)
                        ks = np.arange(M) + i + 1
                    js = np.arange(M)
                    sel = (ks >= 0) & (ks <= 113)
                    band[ks[sel], js[sel]] = kk[i, jk, m]
                if jk == 1:
                    band[114 + m, :] = 1.0
    bands = np.ascontiguousarray(bands.transpose(1, 0, 2).reshape(KP, 12 * M))
    brows = np.empty((MULT, COLS), np.float32)
    for m in range(MULT):
        brows[m] = np.tile(np.asarray(bias, np.float32)[m::MULT], WH + 2)
    return bands, brows


def kernel(**inputs):
    dt = np.float32 if XDT == "f32r" else np.float16
    x = np.ascontiguousarray(np.asarray(inputs["x"]).astype(dt))
    bands, brows = _host_consts(inputs["kernel"], inputs["bias"])
    bands = bands.astype(dt)
    brows = brows.astype(dt)

    if "nc" not in _cache:
        _cache["nc"] = _build()
    nc = _cache["nc"]

    from concourse.bass_utils import run_bass_kernel_spmd

    in_maps = [
        {"x": x[i * BPC : (i + 1) * BPC], "bands": bands, "brows": brows}
        for i in range(NCORES)
    ]
    res = run_bass_kernel_spmd(nc, in_maps, list(range(NCORES)))
    return np.concatenate(
        [res.results[i]["out"].astype(np.float32) for i in range(NCORES)], axis=0
    )
